# revision 26
# baseline (speedup 1.0000x reference)
"""GQA attention kernel for 8 TRN2 NeuronCores (Bass/Tile, SPMD).

Sharding: core c -> (batch b = c // 4, kv-head kv = c % 4). Each core computes
the 4 query heads of its kv group for its batch and a partial (transposed)
output projection; the host sums the 4 partials per batch.

v2: fp16 everywhere on-chip (f32 PSUM accumulate), softmax denominators via
DVE-accumulated P_sum + gpsimd partition_all_reduce (no ones-matmuls), causal
diagonal trimmed to 128-column granularity, and software-pipelined emission:
projection chains for chunk j+1 and output-projection blocks for chunk j-1
are interleaved into attention phase j so the (in-order) PE queue never
stalls on the ACT-paced exp pipeline. Weights ride the sync HWDGE ring,
x-chunks the scalar HWDGE ring, constants the gpsimd SWDGE ring.
"""

import os
import sys

import numpy as np

for _p in ("/opt/trn_rl_repo", "/root/.axon_site/_ro/trn_rl_repo"):
    if os.path.isdir(_p) and _p not in sys.path:
        sys.path.insert(0, _p)

import concourse.bass as bass  # noqa: E402
import concourse.bass_isa as bass_isa  # noqa: E402
import concourse.mybir as mybir  # noqa: E402
from concourse import bacc  # noqa: E402
from concourse.tile import TileContext  # noqa: E402
from concourse.bass_utils import run_bass_kernel_spmd  # noqa: E402

B, T, D = 2, 2048, 2048
H, HKV, HD = 16, 4, 128
G = H // HKV            # query heads per kv head (= per core)
EQ = G * HD             # 512: query-projection rows per core
P = 128
TC = 512                # t-chunk (free dim of every matmul)
NJ = T // TC            # 4 chunks
DT = D // P             # 16 contraction tiles
DEPTH = 3               # exp/mask run this many S-tiles ahead of PV
SCALE = 1.0 / float(np.sqrt(HD))

F32 = mybir.dt.float32
F16 = mybir.dt.float16
EXP = mybir.ActivationFunctionType.Exp
RADD = bass_isa.ReduceOp.add

_CACHE = {}


def _build():
    nc = bacc.Bacc("TRN2", target_bir_lowering=False, debug=False)

    # All inputs arrive pre-transposed into SBUF layout (partition dim first,
    # contiguous per partition) so every DMA runs at full descriptor rate.
    xT = nc.declare_dram_parameter("xT", [P, NJ, 4, 4, TC], F16, isOutput=False)
    wqT = nc.declare_dram_parameter("wqT", [P, G, DT, HD], F16, isOutput=False)
    wkT = nc.declare_dram_parameter("wkT", [P, DT, HD], F16, isOutput=False)
    wvT = nc.declare_dram_parameter("wvT", [P, DT, HD], F16, isOutput=False)
    woT = nc.declare_dram_parameter("woT", [P, G, D], F16, isOutput=False)
    cosT = nc.declare_dram_parameter("cosT", [HD, T], F16, isOutput=False)
    sinT = nc.declare_dram_parameter("sinT", [HD, T], F16, isOutput=False)
    rmat = nc.declare_dram_parameter("rmat", [HD, HD], F16, isOutput=False)
    iden = nc.declare_dram_parameter("iden", [P, P], F16, isOutput=False)
    maskt = nc.declare_dram_parameter("maskt", [P, P], F16, isOutput=False)
    ones_k = nc.declare_dram_parameter("ones_k", [P, 1], F16, isOutput=False)
    yT = nc.declare_dram_parameter("yT", [D, T], F16, isOutput=True)

    with TileContext(nc) as tc:
        with (
            tc.tile_pool(name="const", bufs=1) as cst,
            tc.tile_pool(name="kv", bufs=1) as kvp,
            tc.tile_pool(name="ot", bufs=1) as otp,
            tc.tile_pool(name="wts", bufs=1) as wts,
            tc.tile_pool(name="xs", bufs=2) as xs,
            tc.tile_pool(name="qk", bufs=2) as qk,
            tc.tile_pool(name="vt", bufs=2) as vtp,
            tc.tile_pool(name="work", bufs=5) as wk,
            tc.tile_pool(name="rtmp", bufs=2) as rtmp,
            tc.tile_pool(name="ls", bufs=2) as lsp,
            tc.tile_pool(name="yout", bufs=6) as yop,
            tc.tile_pool(name="ps_acc", bufs=2, space="PSUM") as ps_acc,
            tc.tile_pool(name="ps_s", bufs=2, space="PSUM") as ps_s,
            tc.tile_pool(name="ps_o", bufs=2, space="PSUM") as ps_o,
            tc.tile_pool(name="ps_y", bufs=2, space="PSUM") as ps_y,
        ):
            # Constants ride the gpsimd SWDGE ring so they don't delay the
            # weight/x loads on the two HWDGE rings.
            cos_sb = cst.tile([HD, T], F16, tag="cos")
            sin_sb = cst.tile([HD, T], F16, tag="sin")
            rmat_sb = cst.tile([HD, HD], F16, tag="rmat")
            iden_sb = cst.tile([P, P], F16, tag="iden")
            mask_sb = cst.tile([P, P], F16, tag="mask")
            onek_sb = cst.tile([P, 1], F16, tag="onek")
            # tiny constants first; cos/sin split per chunk so chunk-0 rope
            # only waits on its own slice
            nc.gpsimd.dma_start(rmat_sb[:], rmat[:])
            nc.gpsimd.dma_start(iden_sb[:], iden[:])
            nc.gpsimd.dma_start(mask_sb[:], maskt[:])
            nc.gpsimd.dma_start(onek_sb[:], ones_k[:])
            for c in range(NJ):
                csl = slice(c * TC, (c + 1) * TC)
                nc.gpsimd.dma_start(cos_sb[:, csl], cosT[:, csl])
                nc.gpsimd.dma_start(sin_sb[:, csl], sinT[:, csl])

            # Per-chunk K/V/attn-out tiles (separate tiles per chunk so the
            # interleaved emission never creates false whole-tile hazards
            # between phase B_j reads and phase A_{j+1} writes).
            kt_sbs = [kvp.tile([HD, TC], F16, tag=f"kt{c}", name=f"kt{c}")
                      for c in range(NJ)]
            v_sbs = [kvp.tile([P, 4, HD], F16, tag=f"v{c}", name=f"v{c}")
                     for c in range(NJ)]
            ot_js = [otp.tile([HD, G, TC], F16, tag=f"ot{c}", name=f"ot{c}")
                     for c in range(NJ)]

            wq_sb = wts.tile([P, G, DT, HD], F16, tag="wq")
            wk_sb = wts.tile([P, DT, HD], F16, tag="wk")
            wv_sb = wts.tile([P, DT, HD], F16, tag="wv")
            wo_sb = wts.tile([P, G, D], F16, tag="wo")

            def load_x(j):
                # x chunk j quarters on the scalar HWDGE ring (weights keep
                # the sync ring to themselves; mixing rings creates false
                # cross-ring completion-semaphore ordering).
                tiles = []
                for q in range(4):
                    xq = xs.tile([P, 4, TC], F16, tag=f"xc{q}", name=f"xc{q}")
                    nc.scalar.dma_start(xq[:], xT[:, j, q])
                    tiles.append(xq)
                return tiles



            def psum_s(shape=(P, TC), dtype=F32):
                return ps_s.tile(list(shape), dtype, tag="s", name="s")

            def finish_rope(s, t1, jsl):
                # s <- s*cos + rotate_half(s)*sin; t1 = s*cos precomputed
                pr = psum_s()
                nc.tensor.matmul(pr[:], rmat_sb[:], s, start=True, stop=True)
                nc.vector.tensor_mul(out=s, in0=pr[:], in1=sin_sb[:, jsl])
                nc.vector.tensor_add(out=s, in0=s, in1=t1[:])

            def a_thunks(j, xcq, loader=None):
                """Emission thunks for phase A_j: 6 projection chains
                (K, Q0..Q3, V) + rope + V transposes, each a callable.
                `loader(a, dt)` (chunk 0) emits each weight/x DMA right
                before its consuming matmul so the conservative per-lane
                completion waits stay tight."""
                jsl = slice(j * TC, (j + 1) * TC)
                qt = qk.tile([HD, G, TC], F16, tag="qt")
                vt = vtp.tile([HD, TC], F16, tag="vt")
                rope_q = []

                def chain(a):
                    def emit():
                        acc = ps_acc.tile([P, TC], F32, tag="acc", name="acc")
                        for dt in range(DT):
                            if loader is not None:
                                loader(a, dt)
                            if a == 0:
                                lhsT = wk_sb[:, dt]
                            elif a == 5:
                                lhsT = wv_sb[:, dt]
                            else:
                                lhsT = wq_sb[:, a - 1, dt]
                            nc.tensor.matmul(acc[:], lhsT,
                                             xcq[dt // 4][:, dt % 4],
                                             start=(dt == 0),
                                             stop=(dt == DT - 1))
                        if a == 5:
                            nc.scalar.copy(vt[:], acc[:])
                        else:
                            s = kt_sbs[j][:] if a == 0 else qt[:, a - 1]
                            nc.scalar.copy(s, acc[:])
                            t1 = rtmp.tile([HD, TC], F16, tag="t1")
                            nc.vector.tensor_mul(out=t1[:], in0=s,
                                                 in1=cos_sb[:, jsl])
                            rope_q.append((s, t1))
                        if a >= 1 and rope_q:
                            finish_rope(*rope_q.pop(0), jsl)
                    return emit

                def transposes():
                    while rope_q:
                        finish_rope(*rope_q.pop(0), jsl)
                    for tt in range(4):
                        pvt = psum_s((P, P), F16)
                        nc.tensor.transpose(pvt[:], vt[:, tt * P:(tt + 1) * P],
                                            iden_sb[:])
                        nc.vector.tensor_copy(v_sbs[j][:, tt], pvt[:])

                return [chain(a) for a in range(6)] + [transposes], qt

            def c_thunks(j):
                """Emission thunks for phase C_j: output projection of
                attention chunk j, one thunk per 128-row output block."""
                jsl = slice(j * TC, (j + 1) * TC)

                last = j == NJ - 1

                def block(dt):
                    def emit():
                        # in the tail (no attention running) rotate across
                        # all three idle psum pools for a 6-deep pipeline
                        pool, tg = ([(ps_y, "y"), (ps_s, "s"),
                                     (ps_acc, "acc")][dt % 3]
                                    if last else (ps_y, "y"))
                        py = pool.tile([P, TC], F32, tag=tg, name="py")
                        for g in range(G):
                            nc.tensor.matmul(py[:],
                                             wo_sb[:, g, dt * P:(dt + 1) * P],
                                             ot_js[j][:, g],
                                             start=(g == 0), stop=(g == G - 1))
                        y_sb = yop.tile([P, TC], F16, tag="ysb")
                        # alternate the eviction engine so neither ACT (exp)
                        # nor DVE becomes the pacer
                        if dt % 2 == 0:
                            nc.vector.tensor_copy(y_sb[:], py[:])
                        else:
                            nc.scalar.copy(y_sb[:], py[:])
                        nc.sync.dma_start(yT[dt * P:(dt + 1) * P, jsl], y_sb[:])
                    return emit

                return [block(dt) for dt in range(DT)]

            def emit_b(j, qt, fillers):
                """Attention for q-block j (all 4 heads), with `fillers`
                (independent emission thunks) woven in so the PE queue keeps
                streaming while exp paces the softmax pipeline."""
                jsl = slice(j * TC, (j + 1) * TC)
                nk = 4 * (j + 1)
                nfill = len(fillers)
                slots = G * nk
                fi = 0
                done = 0

                for h in range(G):
                    po = ps_o.tile([P, TC], F32, tag="o", name="po")
                    psum16 = lsp.tile([P, TC], F16, tag="psum")
                    pipe = []

                    def drain():
                        ppt, pkt, pqs = pipe.pop(0)
                        nc.tensor.matmul(po[:, pqs], v_sbs[pkt // 4][:, pkt % 4],
                                         ppt[:, pqs],
                                         start=(pkt == 0), stop=(pkt == nk - 1))

                    for kt in range(nk):
                        m = kt - 4 * j
                        off = 0 if m < 0 else P * m
                        qs = slice(off, TC)
                        pss = psum_s()
                        c, q = kt // 4, kt % 4
                        nc.tensor.matmul(pss[:, qs],
                                         kt_sbs[c][:, q * P:(q + 1) * P],
                                         qt[:, h, qs], start=True, stop=True)
                        pt = wk.tile([P, TC], F16, tag="pt")
                        nc.scalar.activation(pt[:, qs], pss[:, qs], EXP,
                                             scale=SCALE)
                        if m >= 0:
                            ssl = slice(off, off + P)
                            nc.vector.tensor_mul(out=pt[:, ssl], in0=pt[:, ssl],
                                                 in1=mask_sb[:])
                        if kt == 0:
                            nc.vector.tensor_copy(psum16[:], pt[:])
                        else:
                            nc.vector.tensor_add(out=psum16[:, qs],
                                                 in0=psum16[:, qs],
                                                 in1=pt[:, qs])
                        pipe.append((pt, kt, qs))
                        if len(pipe) > DEPTH:
                            drain()
                        done += 1
                        want = nfill * done // slots
                        while fi < want:
                            fillers[fi]()
                            fi += 1
                    while pipe:
                        drain()
                    # softmax denominator: ones-matmul over P_sum, then
                    # reciprocal + partition broadcast + scale.
                    pl = ps_acc.tile([1, TC], F32, tag="acc", name="pl")
                    nc.tensor.matmul(pl[:], onek_sb[:], psum16[:],
                                     start=True, stop=True)
                    rinv = lsp.tile([1, TC], F32, tag="rinv")
                    nc.vector.reciprocal_approx_fast(rinv[:], pl[:])
                    binv = lsp.tile([P, TC], F32, tag="binv")
                    nc.gpsimd.partition_broadcast(binv[:], rinv[:])
                    nc.vector.tensor_mul(out=ot_js[j][:, h], in0=po[:],
                                         in1=binv[:])
                while fi < nfill:
                    fillers[fi]()
                    fi += 1

            # ---- emission schedule -------------------------------------
            # A_0 streams in at 0.125 MiB granularity: every weight quarter
            # and x piece is emitted immediately before its consuming
            # matmul. DMA-completion waits are conservative per semaphore
            # lane (a consumer waits for everything emitted before it on
            # its lanes), so tight interleave keeps the first chains from
            # waiting on later DMAs.
            xcq = [xs.tile([P, 4, TC], F16, tag=f"xc{q}", name=f"xc{q}")
                   for q in range(4)]

            def loader0(a, dt):
                q, dtq = dt // 4, dt % 4
                if a == 0:
                    if dtq == 0:
                        nc.sync.dma_start(wk_sb[:, 4 * q:4 * q + 4],
                                          wkT[:, 4 * q:4 * q + 4])
                    nc.scalar.dma_start(xcq[q][:, dtq], xT[:, 0, q, dtq])
                elif dtq == 0:
                    if a == 5:
                        nc.sync.dma_start(wv_sb[:, 4 * q:4 * q + 4],
                                          wvT[:, 4 * q:4 * q + 4])
                    else:
                        nc.sync.dma_start(wq_sb[:, a - 1, 4 * q:4 * q + 4],
                                          wqT[:, a - 1, 4 * q:4 * q + 4])

            athk, qt = a_thunks(0, xcq, loader=loader0)
            for t in athk:
                t()
            for g in range(G):
                nc.sync.dma_start(wo_sb[:, g], woT[:, g])
            for j in range(NJ):
                fillers = []
                if j + 1 < NJ:
                    xcq = load_x(j + 1)
                    athk, qt_next = a_thunks(j + 1, xcq)
                    fillers += athk
                else:
                    qt_next = None
                if j > 0:
                    fillers += c_thunks(j - 1)
                emit_b(j, qt, fillers)
                qt = qt_next
            for t in c_thunks(NJ - 1):
                t()

    nc.compile()
    return nc


def _host_shards(inputs):
    x = np.asarray(inputs["x"], dtype=np.float32)
    cos = np.asarray(inputs["cos"], dtype=np.float32)
    sin = np.asarray(inputs["sin"], dtype=np.float32)
    Wq = np.asarray(inputs["Wq"], dtype=np.float32)
    Wk = np.asarray(inputs["Wk"], dtype=np.float32)
    Wv = np.asarray(inputs["Wv"], dtype=np.float32)
    Wo = np.asarray(inputs["Wo"], dtype=np.float32)

    f16 = np.float16
    cosT = np.ascontiguousarray(cos.T).astype(f16)
    sinT = np.ascontiguousarray(sin.T).astype(f16)
    rmat = np.zeros((HD, HD), f16)
    half = HD // 2
    for i in range(half):
        rmat[i + half, i] = -1.0     # out[m<64] = -q[m+64]
        rmat[i, i + half] = 1.0      # out[m>=64] = q[m-64]
    iden = np.eye(P, dtype=f16)
    # one lower-triangle mask block reused for every diagonal k-tile
    maskt = (np.arange(P)[None, :] >= np.arange(P)[:, None]).astype(f16)
    ones_k = np.ones((P, 1), f16)

    def to_sbuf_layout(wT, cols):
        # [D_contract, cols] -> [P, D_contract//P, cols], partition dim first
        return np.ascontiguousarray(
            wT.reshape(-1, P, cols).transpose(1, 0, 2)).astype(f16)

    # x[b].T is [d, t]; device layout [p, j, q, dtq, t'] with d = (4q+dtq)*P+p
    # and t = j*TC + t' makes each (j, q) quarter-load fully contiguous.
    xTs = [np.ascontiguousarray(
        x[b].T.reshape(4, 4, P, NJ, TC).transpose(2, 3, 0, 1, 4)).astype(f16)
        for b in range(B)]
    wqTs = []
    for kv in range(HKV):
        per_h = [to_sbuf_layout(
            Wq[kv * EQ + h * HD: kv * EQ + (h + 1) * HD].T, HD)
            for h in range(G)]
        wqTs.append(np.ascontiguousarray(np.stack(per_h, axis=1)))
    wkTs = [to_sbuf_layout(Wk[kv * HD:(kv + 1) * HD].T, HD) for kv in range(HKV)]
    wvTs = [to_sbuf_layout(Wv[kv * HD:(kv + 1) * HD].T, HD) for kv in range(HKV)]
    woTs = [to_sbuf_layout(Wo[:, kv * EQ:(kv + 1) * EQ].T, D) for kv in range(HKV)]

    in_maps = []
    for c in range(8):
        b, kv = divmod(c, HKV)
        in_maps.append({
            "xT": xTs[b], "wqT": wqTs[kv], "wkT": wkTs[kv], "wvT": wvTs[kv],
            "woT": woTs[kv], "cosT": cosT, "sinT": sinT, "rmat": rmat,
            "iden": iden, "maskt": maskt, "ones_k": ones_k,
        })
    return in_maps


def get_nc():
    if "nc" not in _CACHE:
        _CACHE["nc"] = _build()
    return _CACHE["nc"]


def run(inputs, **kw):
    nc = get_nc()
    in_maps = _host_shards(inputs)
    res = run_bass_kernel_spmd(nc, in_maps, core_ids=list(range(8)), **kw)
    out = np.zeros((B, T, D), np.float32)
    for c in range(8):
        b = c // HKV
        out[b] += res.results[c]["yT"].astype(np.float32).T
    return out, res


def kernel(**inputs) -> np.ndarray:
    out, _ = run(inputs)
    return out
